# revision 33
# baseline (speedup 1.0000x reference)
"""Multi-head attention on 8 TRN2 NeuronCores.

Problem: queries [B,N,L,H,E], keys [B,N,S,H,E], values [B,N,S,H,D]
         out[b,n,l,h,:] = softmax(Q[b,n,l,h,:] @ K[b,n,:,h,:]^T / sqrt(E)) @ V[b,n,:,h,:]
with B,N,L,S,H,E,D = 4,7,512,512,8,64,64.

Sharding: head-parallel - core c computes all B*N=28 (b,n) slices for head h=c.

Device kernel v2 (all matmul operands fp16; fp32 PSUM accumulate):
  1. QK row-tiled: contraction E=64 only needs half the PE rows, so the
     slice-pair runs CONCURRENTLY on two independent 64x128 PE tiles
     (T0 = SBUF partitions 0-63 = slice a, T8 = partitions 64-127 =
     slice b).  8 matmuls per pair emitted T0/T8-interleaved stream in
     ~4x512 cycles instead of 8x512.
  2. exp split across two engines to beat the ScalarE throughput floor:
     11/16 of [128,1024] score units get a true ScalarE exp ACTIVATE;
     5/16 get a one-op DVE Schraudolph (tensor_scalar fp32->int16:
     bits = rint(1024*log2e*scale*x + 15302), int16 tile bitcast to
     fp16 = 2^f linear-interp exp, ~3% max err; end-to-end rel err
     ~1e-2, sim-validated).
  3. PV untiled (contraction S-chunk=128 uses the full array): po[0:65]
     = [ones|V]^T @ attnT accumulated over 4 s-chunks; row 0 = softmax
     denominator, rows 1-64 = unnormalized numerator^T.  No on-device
     normalization: DVE copies po -> fp16 and the HOST divides
     num/den while unsharding (kills the old recip/GpSimd-broadcast/
     mul epilogue chain and its 10us drain tail).
  4. DMA queues split: inputs issue from the GpSimd queue, outputs from
     Sync, so input prefetch never parks behind output DMAs.

PSUM budget (8 banks exactly): 3 score units [128,1024] (2 banks each,
round-robin; unit u tag u%3) + 2 po [128,512] banks.  PV of pair p-1 is
emitted between QK(p) and QK(p+1) so the PE never head-of-line blocks
on an exp that hasn't run yet.
"""

import numpy as np

B, N, L, S, H, E, D = 4, 7, 512, 512, 8, 64, 64
NS = B * N          # 28 (b,n) slices per core
NP = NS // 2        # 14 slice-pairs
P = 128
SC = S // P         # 4 s-chunks
SCALE = 1.0 / float(np.sqrt(E))

# input pack layout (fp16), per slice-pair: [128, 1544] =
#   [0:512)     qtT pair  (rows 0-63 = slice a's [E, L], rows 64-127 = slice b)
#   [512:1024)  ktT pair  (same row split, cols = S)
#   [1024:1284) VA slice a: 4 s-chunks x 65 cols, each [ones | V_chunk^T]
#   [1284:1544) VA slice b
QOFF, KOFF, VOFF = 0, 512, 1024
VW = D + 1          # 65 cols per VA chunk
IN_COLS = 1024 + 2 * SC * VW  # 1544

# Schraudolph fp16-bits exp: bits = rint(A*x + Bc), viewed as fp16
SCHRAU_A = 1024.0 * np.log2(np.e) * SCALE
SCHRAU_B = 15360.0 - 58.0
# units with (u*5)//16 incrementing go to DVE (5/16 of all units).
# (A deterministic gating-unit assignment and a 6/16 split both measured
# ~0.7us WORSE - the ScalarE waits at QK block starts overlap PE-idle
# that the PE floor imposes anyway, so shifting exp around just moves
# them.)
DVE_NUM, DVE_DEN = 5, 16

_CACHE = {}


def _is_dve_unit(u):
    # u == 1 special case: pair 0's second-emitted exp gates QK(1) during
    # pipeline fill while ScalarE is still cold-starting; DVE clears it
    # ~0.7us sooner.
    return u == 1 or (u * DVE_NUM) // DVE_DEN > ((u - 1) * DVE_NUM) // DVE_DEN


def _build_program():
    import concourse.mybir as mybir
    import concourse.tile as tile
    from concourse import bacc
    import concourse.bass as bass
    from concourse.alu_op_type import AluOpType

    f32 = mybir.dt.float32
    f16 = mybir.dt.float16
    i16 = mybir.dt.int16
    Exp = mybir.ActivationFunctionType.Exp

    nc = bacc.Bacc("TRN2", target_bir_lowering=False, debug=False)
    inp = nc.dram_tensor("inp", [NP, P, IN_COLS], f16, kind="ExternalInput").ap()
    o = nc.dram_tensor("o", [NP, VW, 2 * L], f16, kind="ExternalOutput").ap()

    with tile.TileContext(nc) as tc:
        with (
            tc.tile_pool(name="inpool", bufs=1) as in_pool,
            tc.tile_pool(name="attn", bufs=1) as at_pool,
            tc.tile_pool(name="osb", bufs=1) as osb_pool,
            tc.tile_pool(name="ps", bufs=1, space=bass.MemorySpace.PSUM) as ps_pool,
            tc.tile_pool(name="po", bufs=1, space=bass.MemorySpace.PSUM) as po_pool,
        ):
            # HAM warm-up: tiled dummy matmul pairs (no DMA dependency) open
            # the PE clock gate (1.2 -> 2.4 GHz) while the first input DMA is
            # in flight; the dummy ACTIVATE preloads the exp table set.
            # 24 MMs = 12 concurrent T0/T8 slots ~= 5.1us of continuous PE
            # busy at the cold 1.2 GHz clock - one full free-running
            # 4096-cycle HAM window, so the gate opens before real work.
            # (12 tiled MMs = 2.6us busy never opened it.)  Concurrent row
            # tiles must never share a PSUM bank (HW fatal): T0 warms into
            # the po0 bank, T8 into po1.
            # (warm cannot be read uninitialized - Tile refuses to
            # allocate read-only tiles - so the DVE memset stays.)
            warm = in_pool.tile([P, L], f16, tag="warm", bufs=1)
            nc.vector.memset(warm[:], 1.0)
            dummy = osb_pool.tile([1, 2], f32, tag="dummy")
            nc.scalar.activation(dummy[:], warm[0:1, 0:2], Exp, scale=SCALE)
            wps0 = po_pool.tile([P, L], f32, tag="po0")
            wps1 = po_pool.tile([P, L], f32, tag="po1")
            for w in range(16):
                tp = (0, 0) if w % 2 == 0 else (64, 0)
                r0 = 0 if w % 2 == 0 else 64
                nc.tensor.matmul(
                    (wps0 if w % 2 == 0 else wps1)[:],
                    lhsT=warm[r0:r0 + 64, 0:P], rhs=warm[r0:r0 + 64, :],
                    start=True, stop=True, tile_position=tp,
                )

            in_tiles = {}

            def load_pair(p):
                if p < NP and p not in in_tiles:
                    t = in_pool.tile([P, IN_COLS], f16, tag=f"t{p % 5}")
                    if p < 2:
                        # startup critical path: one queue ring moves only
                        # ~110 GB/s, so a whole 396KB pair takes ~3.6us and
                        # the PE idles after warmup.  Split the first pairs
                        # across three rings (Q on gpsimd, K on sync, VA on
                        # scalar - VA is only needed ~4us later by PV, and
                        # the exp table load can wait behind its issue).
                        # Tile's subtile deps let QK start on Q+K alone.
                        nc.gpsimd.dma_start(t[:, QOFF:QOFF + L], inp[p][:, QOFF:QOFF + L])
                        nc.sync.dma_start(t[:, KOFF:KOFF + S], inp[p][:, KOFF:KOFF + S])
                        nc.scalar.dma_start(t[:, VOFF:], inp[p][:, VOFF:])
                    else:
                        nc.gpsimd.dma_start(t[:], inp[p])
                    in_tiles[p] = t

            load_pair(0)
            load_pair(1)

            # at_units[(pair, j, h)] -> fp16 [128, 1024] attn tile
            at_units = {}

            def emit_qk(p, in_t):
                """Row-tiled QK for pair p: units (j, h) = slice j's s-chunks
                2h, 2h+1; T0/T8-interleaved matmul stream + exp per unit."""
                ps_tiles = {}
                for h in range(2):
                    for j in range(2):
                        # tag by EMISSION order (2h+j), not unit id (2j+h):
                        # the next pair's first QK tile then WARs this
                        # pair's second-emitted exp instead of its third,
                        # gaining ~1us of slack (measured 0.8-1.2us PE
                        # waits on ACTIVATE completion otherwise).
                        eu = 4 * p + 2 * h + j
                        ps_tiles[(j, h)] = ps_pool.tile(
                            [P, 2 * L], f32, tag=f"ps{eu % 3}", name=f"ps_u{eu}")
                    # 64x128 mode, 2 independent tiles: T0 = slice a, T8 =
                    # slice b.  (A 4-tile 64x64 variant cuts LDW cycles but
                    # doubles SBUF moving-operand reads, tripping the
                    # chip-wide P0 power downclock to 2.0 GHz - measured
                    # net loss.  Keep 2 tiles.)
                    for k in range(2):
                        sc = 2 * h + k
                        for j in range(2):
                            nc.tensor.matmul(
                                ps_tiles[(j, h)][:, k * L:(k + 1) * L],
                                lhsT=in_t[j * E:(j + 1) * E, KOFF + sc * P:KOFF + (sc + 1) * P],
                                rhs=in_t[j * E:(j + 1) * E, QOFF:QOFF + L],
                                start=True, stop=True,
                                tile_position=(64 * j, 0),
                            )
                    # both slices' chunk-pair h done -> exp the two units
                    for j in range(2):
                        u = 4 * p + 2 * h + j
                        ps = ps_tiles[(j, h)]
                        if _is_dve_unit(u):
                            ati = at_pool.tile([P, 2 * L], i16, tag=f"at{u % 12}")
                            nc.vector.tensor_scalar(
                                out=ati[:], in0=ps[:],
                                scalar1=SCHRAU_A, scalar2=SCHRAU_B,
                                op0=AluOpType.mult, op1=AluOpType.add,
                            )
                            at = ati[:].bitcast(f16)
                        else:
                            atf = at_pool.tile([P, 2 * L], f16, tag=f"at{u % 12}")
                            nc.scalar.activation(atf[:], ps[:], Exp, scale=SCALE)
                            at = atf[:]
                        at_units[(p, j, h)] = at

            def emit_pv(p, in_t):
                """Untiled PV + epilogue for both slices of pair p; the two
                slices share one osb tile so the pair ships as ONE output
                DMA (halves the sync-queue instruction count)."""
                osb = osb_pool.tile([VW, 2 * L], f16, tag=f"o{p % 2}", name=f"osb{p}")
                for j in range(2):
                    po = po_pool.tile([P, L], f32, tag=f"po{j}", name=f"po{2 * p + j}")
                    for sc in range(SC):
                        at = at_units[(p, j, sc // 2)]
                        nc.tensor.matmul(
                            po[0:VW, :],
                            lhsT=in_t[:, VOFF + j * SC * VW + sc * VW: VOFF + j * SC * VW + (sc + 1) * VW],
                            rhs=at[:, (sc % 2) * L:(sc % 2 + 1) * L],
                            start=(sc == 0),
                            stop=(sc == SC - 1),
                        )
                    del at_units[(p, j, 0)], at_units[(p, j, 1)]
                    if p == NP - 1 and j == 1:
                        # last slice: ScalarE (idle by now) does the PSUM
                        # evacuation so the kernel tail doesn't queue
                        # behind the DVE's remaining work (~0.9us saved).
                        nc.scalar.copy(osb[:, j * L:(j + 1) * L], po[0:VW, :])
                    else:
                        nc.vector.tensor_copy(osb[:, j * L:(j + 1) * L], po[0:VW, :])
                nc.sync.dma_start(o[p], osb[:])

            # PV(p-1) is emitted between QK(p) and QK(p+1): the PE is
            # in-order, so PV must only ever wait on exps that were queued
            # a full pair earlier (no head-of-line blocking).  (2-pair
            # QK/PV batching to halve the tiling-mode switches measured
            # 2.3us WORSE - deeper WAR coupling beats the switch savings.)
            def pad(n):
                # dummy tiled warm MMs that execute during what would be
                # PE idle in the pipeline-fill region (waiting on the
                # first exps): they keep the HAM MID idle-detector from
                # re-throttling the clock right after it opens (observed
                # bounce: open at 12.2us, re-throttle 15.6-29.3us at half
                # clock on unlucky phases).  Same 64-row mode as QK (no
                # extra mode switch); po banks are overwritten by PV's
                # start=True later.
                for w in range(2 * n):
                    tp = (0, 0) if w % 2 == 0 else (64, 0)
                    r0 = 0 if w % 2 == 0 else 64
                    nc.tensor.matmul(
                        (wps0 if w % 2 == 0 else wps1)[:],
                        lhsT=warm[r0:r0 + 64, 0:P], rhs=warm[r0:r0 + 64, :],
                        start=True, stop=True, tile_position=tp,
                    )

            prev = None
            for pair in range(NP):
                in_t = in_tiles[pair]
                load_pair(pair + 2)
                emit_qk(pair, in_t)
                if pair == 0:
                    pad(4)
                elif pair == 1:
                    pad(2)
                if prev is not None:
                    p_prev, t_prev = prev
                    emit_pv(p_prev, t_prev)
                    del in_tiles[p_prev]
                prev = (pair, in_t)
            emit_pv(prev[0], prev[1])
    nc.compile()
    return nc


def _prep_inputs(queries, keys, values):
    """Pack per-core fp16 inputs. Core c gets head h=c."""
    q = np.asarray(queries, dtype=np.float32)
    k = np.asarray(keys, dtype=np.float32)
    v = np.asarray(values, dtype=np.float32)

    # [H, NP, 128, 512] - Q^T/K^T per slice, slice-pairs stacked on partitions
    qt = np.ascontiguousarray(q.transpose(3, 0, 1, 4, 2)).reshape(H, NP, P, L)
    kt = np.ascontiguousarray(k.transpose(3, 0, 1, 4, 2)).reshape(H, NP, P, S)

    # VA: [H, NS, SC, 128 s, 65] = [ones | V_chunk] -> [H, NP, 128, 2*SC*65]
    va = np.empty((H, NS, SC, P, VW), dtype=np.float32)
    va[..., 0] = 1.0
    va[..., 1:] = v.transpose(3, 0, 1, 2, 4).reshape(H, NS, SC, P, D)
    va = va.transpose(0, 1, 3, 2, 4).reshape(H, NP, 2, P, SC * VW)
    va = np.ascontiguousarray(va.transpose(0, 1, 3, 2, 4)).reshape(H, NP, P, 2 * SC * VW)

    inp = np.concatenate([qt, kt, va], axis=-1).astype(np.float16)
    return [{"inp": inp[c]} for c in range(H)]


def _run(in_maps, trace=False, tmpdir=None):
    from concourse.bass_utils import run_bass_kernel_spmd

    if "nc" not in _CACHE:
        _CACHE["nc"] = _build_program()
    kwargs = {}
    if tmpdir is not None:
        kwargs["tmpdir"] = tmpdir
    return run_bass_kernel_spmd(
        _CACHE["nc"], in_maps, core_ids=list(range(H)), trace=trace, **kwargs
    )


def kernel(queries, keys, values, _trace=False, _results_out=None, _tmpdir=None):
    in_maps = _prep_inputs(queries, keys, values)
    res = _run(in_maps, trace=_trace, tmpdir=_tmpdir)
    if _results_out is not None:
        _results_out.append(res)
    # res.results[c]["o"]: [NP, 65, 2*512] fp16; free axis = [slice a | slice b];
    # row 0 = denom, rows 1: = num^T
    oall = np.stack([res.results[c]["o"] for c in range(H)], axis=0).astype(np.float32)
    oall = oall.reshape(H, NP, VW, 2, L).transpose(0, 1, 3, 2, 4).reshape(H, NS, VW, L)
    out = oall[:, :, 1:, :] / oall[:, :, 0:1, :]  # [H, NS, D, L]
    out = out.reshape(H, B, N, D, L).transpose(1, 2, 4, 0, 3)
    return np.ascontiguousarray(out)
